# revision 10
# baseline (speedup 1.0000x reference)
"""Bass/Trainium2 kernel for nn_Attention (additive attention + weighted sum).

Computation (reference):
    enc  = encoder_outputs.transpose(1, 0, 2)              # [B, S, E]
    z    = enc @ w_e.T + hidden @ w_h.T + attn_b           # [B, S, O]
    att  = softmax(tanh(z) @ v, axis=S)                    # [B, S]
    out  = att @ enc                                       # [B, E]

Sharding: data-parallel over batch — 8 cores x 4 batches each.
Host precomputes hidden @ w_h.T + attn_b (0.1% of FLOPs) and ships the
encoder slice in [b, e, s] layout (contraction over e needs e on
partitions for the big matmul).

The PE runs the energy matmuls (64 per 512-chunk) plus ONE 1-row
ones-matmul per chunk; everything else rides the other engines, one
chunk behind the PE:
  ACT:  tanh(psum + bias) -> sbuf energies; chain -> f32r copy; exp
  DVE:  score chain  s[p,t] += v[p+128m] * tanh_m[p,t]
        (scalar_tensor_tensor), then the weighted partial sums
  Pool: broadcast of the exp'd score row
Scores stay small (|s| < ~40 for randn-scale inputs), so exp runs
without the usual running-max shift and the per-batch recombine is a
plain sum of chunk partials over a shared denominator. The encoder is
read from HBM exactly once; PE roofline is ~222us/core.
"""

import numpy as np
from contextlib import ExitStack

# Problem shapes (hardcoded; kernel.py must be self-contained).
B = 32
S = 2048
E = 1024  # encoder hidden
O = 1024  # output dim / attention proj dim
N_CORES = 8
BL = B // N_CORES  # batches per core = 4

P = 128    # partitions
F = 512    # matmul moving free dim (one fp32 PSUM bank)
KE = E // P   # 8 contraction tiles over e
MT = O // P   # 8 output-row tiles over p
F2 = F // 2
# Batches 0..2 run uniform 512-chunks; the last batch ends 256+256 so
# only a half-size post-processing chain is exposed after the final
# matmul.
CHUNKS_STD = [(0, F), (F, F), (2 * F, F), (3 * F, F)]
CHUNKS_LAST = [(0, F), (F, F), (2 * F, F), (3 * F, F2), (3 * F + F2, F2)]
V = len(CHUNKS_LAST)

_PROGRAM = None


def _build_program():
    import concourse.tile as tile
    from concourse import bacc, mybir

    f32 = mybir.dt.float32
    f32r = mybir.dt.float32r
    AF = mybir.ActivationFunctionType
    AX = mybir.AxisListType
    ALU = mybir.AluOpType

    nc = bacc.Bacc("TRN2", target_bir_lowering=False, debug=False,
                   num_devices=N_CORES)

    encT = nc.dram_tensor("encT", [BL, E, S], f32r, kind="ExternalInput").ap()
    weT = nc.dram_tensor("weT", [P, KE, O], f32r, kind="ExternalInput").ap()
    hb = nc.dram_tensor("hb", [P, MT, BL], f32, kind="ExternalInput").ap()
    vm = nc.dram_tensor("vm", [P, MT], f32, kind="ExternalInput").ap()
    onesr = nc.dram_tensor("onesr", [P, 1], f32r, kind="ExternalInput").ap()
    wz = nc.dram_tensor("wz", [P, F], f32r, kind="ExternalInput").ap()
    # out[b, ep, kt] = weighted[b, kt*128 + ep]; host transposes back.
    out = nc.dram_tensor("out", [BL, P, KE], f32, kind="ExternalOutput").ap()

    with tile.TileContext(nc) as tc, ExitStack() as ctx:
        consts = ctx.enter_context(tc.tile_pool(name="consts", bufs=1))
        enc_pool = ctx.enter_context(tc.tile_pool(name="enc", bufs=4))
        epool = ctx.enter_context(tc.tile_pool(name="energy", bufs=10))
        chpool = ctx.enter_context(tc.tile_pool(name="chain", bufs=4))
        crpool = ctx.enter_context(tc.tile_pool(name="chainr", bufs=2))
        erpool = ctx.enter_context(tc.tile_pool(name="erow", bufs=2))
        bpool = ctx.enter_context(tc.tile_pool(name="bcast", bufs=3))
        jpool = ctx.enter_context(tc.tile_pool(name="junk", bufs=2))
        acpool = ctx.enter_context(tc.tile_pool(name="acc", bufs=2))
        opool = ctx.enter_context(tc.tile_pool(name="outsb", bufs=2))
        small = ctx.enter_context(tc.tile_pool(name="small", bufs=10))
        pps = ctx.enter_context(tc.tile_pool(name="pps", bufs=8, space="PSUM"))

        def ps_tile():
            return pps.tile([P, F], f32, tag="ps", name="ps")

        weT_sb = consts.tile([P, KE, O], f32r)
        vm_sb = consts.tile([P, MT], f32)
        hb_sb = consts.tile([P, MT, BL], f32)
        ones_sb = consts.tile([P, 1], f32r)

        def load_chunk(b, lo, ln):
            # one contiguous [P, KE, F] tile per s-chunk: slice k feeds the
            # matmuls; the whole tile feeds the DVE weighted sum.
            t = enc_pool.tile([P, KE, F], f32r, tag="ech")
            for k in range(KE):
                nc.sync.dma_start(
                    t[:, k, :ln], encT[b, k * P:(k + 1) * P, lo:lo + ln])
            return t

        class BState:
            pass

        def b_begin(b):
            st = BState()
            st.denrow = small.tile([1, V], f32, tag="denrow", name="denrow")
            st.acc = acpool.tile([P, KE, V], f32, tag="acc", name="acc")
            return st

        class Pending:
            """Deferred post-work for a chunk: runs interleaved with the
            NEXT chunk's energy matmuls so the PE never waits on it."""

            def __init__(self, st, echunk, chr_, vc, ln):
                self.st, self.echunk, self.chr = st, echunk, chr_
                self.vc, self.ln = vc, ln

        def post_a(st, energies, ln):
            # score chain v.tanh over the 8 m-tiles on DVE, then an ACT
            # copy to f32r so the PE ones-matmul may consume it.
            ch = [chpool.tile([P, F], f32, tag="chain", name="chain")
                  for _ in range(2)]
            nc.vector.tensor_scalar_mul(
                ch[0][:, :ln], energies[0][:, :ln], vm_sb[:, 0:1])
            for m in range(1, MT):
                src, dst = ch[(m + 1) % 2], ch[m % 2]
                nc.vector.scalar_tensor_tensor(
                    dst[:, :ln], energies[m][:, :ln], vm_sb[:, m:m + 1],
                    src[:, :ln], ALU.mult, ALU.add)
            chl = ch[(MT - 1) % 2]
            chr_ = crpool.tile([P, F], f32r, tag="chainr", name="chainr")
            nc.scalar.activation(chr_[:, :ln], chl[:, :ln], AF.Copy)
            return chr_

        def post_b(pend):
            st, echunk, vc, ln = pend.st, pend.echunk, pend.vc, pend.ln
            # partition-sum of the chain on the PE (1-row ones-matmul),
            # exp with running denominator, broadcast, weighted sums.
            sps = ps_tile()
            nc.tensor.matmul(sps[:1, :ln], ones_sb[:], pend.chr[:, :ln],
                             start=True, stop=True)
            erow = erpool.tile([1, F], f32, tag="erow", name="erow")
            nc.scalar.activation(erow[:, :ln], sps[:1, :ln], AF.Exp,
                                 accum_out=st.denrow[:, vc:vc + 1])
            erow_bc = bpool.tile([P, F], f32, tag="erow_bc", name="erow_bc")
            nc.gpsimd.partition_broadcast(erow_bc[:, :ln], erow[:, :ln])
            prod = jpool.tile([P, KE, F], f32, tag="junk", name="prod")
            nc.vector.tensor_tensor(
                prod[:, :, :ln], echunk[:, :, :ln].bitcast(f32),
                erow_bc[:, None, :ln].to_broadcast((P, KE, ln)),
                ALU.mult)
            nc.vector.reduce_sum(st.acc[:, :, vc], prod[:, :, :ln],
                                 axis=AX.X)

        def chunk_compute(b, vc, st, echunk, ln, pending):
            energies = []
            for m in range(MT):
                ps = ps_tile()
                for k in range(KE):
                    nc.tensor.matmul(
                        ps[:, :ln], weT_sb[:, k, m * P:(m + 1) * P],
                        echunk[:, k, :ln], start=(k == 0), stop=(k == KE - 1))
                energy = epool.tile([P, F], f32, tag="energy")
                nc.scalar.activation(energy[:, :ln], ps[:, :ln], AF.Tanh,
                                     bias=hb_sb[:, m, b:b + 1])
                energies.append(energy)
                if m == 0 and pending is not None:
                    post_b(pending)
            chr_ = post_a(st, energies, ln)
            return Pending(st, echunk, chr_, vc, ln)

        def b_end(b, st, nv):
            # recombine: out = (sum_c acc_c) / (sum_c den_c) — no max
            # shift needed (|score| stays far below exp overflow).
            den = small.tile([1, 1], f32, tag="den", name="den")
            nc.vector.reduce_sum(den[:], st.denrow[:, :nv], axis=AX.X)
            rden = small.tile([1, 1], f32, tag="rden", name="rden")
            nc.vector.reciprocal(rden[:], den[:])
            rden_bc = bpool.tile([P, 1], f32, tag="rden_bc", name="rden_bc")
            nc.gpsimd.partition_broadcast(rden_bc[:], rden[:])
            accf = acpool.tile([P, KE], f32, tag="accf", name="accf")
            nc.vector.reduce_sum(accf[:], st.acc[:, :, :nv], axis=AX.X)
            osb = opool.tile([P, KE], f32, tag="osb", name="osb")
            nc.scalar.activation(osb[:], accf[:], AF.Copy, scale=rden_bc[:])
            nc.sync.dma_start(out[b], osb[:])

        # Hybrid PE warm-up: two fp32 matmuls on memset tiles start as
        # soon as the PE queue boots (no DMA dependency), then f32r
        # matmuls on the zeros input keep the clock gate open until real
        # data lands.
        wa = consts.tile([P, P], f32)
        nc.vector.memset(wa[:], 0.0)
        wz_sb = consts.tile([P, F], f32r)
        nc.sync.dma_start(wz_sb[:], wz[:])
        wps = ps_tile()
        for _ in range(2):
            nc.tensor.matmul(wps[:, :P], wa[:], wa[:], start=True, stop=True)
        for _ in range(7):
            nc.tensor.matmul(wps[:], wz_sb[:, :P], wz_sb[:],
                             start=True, stop=True)

        # Startup: weights ride the GpSimd DMA queue so the Sync queue
        # delivers the first encoder chunk immediately.
        ech0 = enc_pool.tile([P, KE, F], f32r, tag="ech")
        for k in range(KE):
            nc.sync.dma_start(
                ech0[:, k, :], encT[0, k * P:(k + 1) * P, 0:F])
            if k == 0:
                for m in range(MT):
                    nc.gpsimd.dma_start(weT_sb[:, 0, m * P:(m + 1) * P],
                                        weT[:, 0, m * P:(m + 1) * P])
            else:
                nc.gpsimd.dma_start(weT_sb[:, k, :], weT[:, k, :])
        nc.gpsimd.dma_start(vm_sb[:], vm[:])
        nc.gpsimd.dma_start(hb_sb[:], hb[:])
        nc.gpsimd.dma_start(ones_sb[:], onesr[:])

        # First chunk: k=0 row across all 8 banks first (needs only
        # weT[k0] + ech0[k0]), then m-major so bank m stops early and
        # tanh/psum recycling pipelines into chunk 1.
        st0 = b_begin(0)
        pstiles = [ps_tile() for _ in range(MT)]
        for m in range(MT):
            nc.tensor.matmul(
                pstiles[m][:], weT_sb[:, 0, m * P:(m + 1) * P],
                ech0[:, 0, :], start=True, stop=False)
        energies0 = []
        for m in range(MT):
            for k in range(1, KE):
                nc.tensor.matmul(
                    pstiles[m][:], weT_sb[:, k, m * P:(m + 1) * P],
                    ech0[:, k, :], start=False, stop=(k == KE - 1))
            energy = epool.tile([P, F], f32, tag="energy")
            nc.scalar.activation(energy[:], pstiles[m][:], AF.Tanh,
                                 bias=hb_sb[:, m, 0:1])
            energies0.append(energy)
        chr0 = post_a(st0, energies0, F)
        pending = Pending(st0, ech0, chr0, 0, F)

        states = {0: st0}
        end_after = None  # (b, st, nv) to finish after next post_b
        for vc in range(1, len(CHUNKS_STD)):
            lo, ln = CHUNKS_STD[vc]
            pending = chunk_compute(0, vc, st0, load_chunk(0, lo, ln), ln,
                                    pending)
        for b in range(1, BL):
            chunks = CHUNKS_LAST if b == BL - 1 else CHUNKS_STD
            states[b] = b_begin(b)
            for vc in range(len(chunks)):
                lo, ln = chunks[vc]
                prev_pend = pending
                pending = chunk_compute(b, vc, states[b],
                                        load_chunk(b, lo, ln), ln, prev_pend)
                if vc == 0:
                    # prev batch's last chunk post just ran inside
                    # chunk_compute; now close out the batch.
                    b_end(b - 1, states.pop(b - 1), len(CHUNKS_STD))
        post_b(pending)
        b_end(BL - 1, states.pop(BL - 1), len(CHUNKS_LAST))

    nc.compile()
    return nc


def _get_program():
    global _PROGRAM
    if _PROGRAM is None:
        _PROGRAM = _build_program()
    return _PROGRAM


def _make_in_maps(hidden, encoder_outputs, attn_w, attn_b, v):
    hidden = np.asarray(hidden, dtype=np.float32)
    enc = np.asarray(encoder_outputs, dtype=np.float32)
    attn_w = np.asarray(attn_w, dtype=np.float32)
    attn_b = np.asarray(attn_b, dtype=np.float32)
    v = np.asarray(v, dtype=np.float32)

    hb_full = hidden @ attn_w[:, :O].T + attn_b          # [B, O]
    weT = np.ascontiguousarray(
        attn_w[:, O:].T.reshape(KE, P, O).transpose(1, 0, 2))  # [P, KE, O]
    vm = np.ascontiguousarray(v.reshape(MT, P).T)        # [P, MT]

    in_maps = []
    for core in range(N_CORES):
        sl = slice(core * BL, (core + 1) * BL)
        encT_c = np.ascontiguousarray(
            enc[:, sl, :].transpose(1, 2, 0))            # [BL, E, S]
        hb_c = np.ascontiguousarray(
            hb_full[sl].T.reshape(MT, P, BL).transpose(1, 0, 2))  # [P, MT, BL]
        in_maps.append({
            "encT": encT_c,
            "weT": weT,
            "hb": hb_c,
            "vm": vm,
            "onesr": np.ones((P, 1), dtype=np.float32),
            "wz": np.zeros((P, F), dtype=np.float32),
        })
    return in_maps


def run(trace=False, **inputs):
    from concourse.bass_utils import run_bass_kernel_spmd
    nc = _get_program()
    in_maps = _make_in_maps(**inputs)
    res = run_bass_kernel_spmd(nc, in_maps, list(range(N_CORES)), trace=trace)
    # out[b, ep, kt] -> weighted[b, kt*128 + ep]
    outp = np.concatenate(
        [res.results[i]["out"].transpose(0, 2, 1).reshape(BL, O)
         for i in range(N_CORES)], axis=0)
    return outp, res


def kernel(**inputs) -> np.ndarray:
    outp, _ = run(trace=False, **inputs)
    return outp
